# revision 1
# baseline (speedup 1.0000x reference)
"""Causal single-head attention on 8 Trainium2 NeuronCores.

Problem: x[4, 4096, 1024], Wq/Wk/Wv[1024, 64] ->
  out = softmax(causal(Q K^T / 8)) V   per batch, fp32.

Sharding: core i handles batch b = i//2 with query-chunk parity p = i%2
(512-wide query chunks; core p owns global chunks {p, 2+p, 4+p, 6+p}).
Both cores of a pair load the full x[b] (transposed on host to [C, T] so the
contraction dim lands on partitions) and compute full K/V; causal work is
balanced by interleaving query chunks.

The SPMD program is identical on all cores. Parity enters only through data:
  - a per-core additive causal mask buffer [128, 1408]
  - a per-core 0/1 predicate for selecting which projection chunk feeds each
    local query slot (copy_predicated)
On-device layout: scores are computed transposed (S^T[k, q] = K^T.T Q^T per
128x512 block) so softmax'd weights P^T feed the PV matmul directly with k on
partitions; V is augmented with a ones column so row-sums accumulate in the
same PSUM tile as P@V; normalization happens after a PE transpose back to
[q, h] layout.

Matmul operands are stored as float32r (TF32-class, 1 cy/row on the PE vs 4
for fp32; measured ~1.5e-4 matmul rel err). Set IN_DT = float32 for the exact
(4x slower) path.
"""

import numpy as np

import concourse.bacc as bacc
import concourse.mybir as mybir
import concourse.tile as tile
from concourse.bass_utils import run_bass_kernel_spmd

# Problem dims
B, T, C, HS = 4, 4096, 1024, 64
P = 128           # partitions
CH = 512          # query-chunk width
NCH = T // CH     # 8 chunks
NSLOT = NCH // 2  # 4 local query slots per core
CSUB = C // P     # 8 contraction subtiles
KT_PER_CH = CH // P   # 4 key tiles (128) per chunk
NKT = T // P      # 32 key tiles total
MASK_W = 896 + CH     # mask buffer width
NEG = -1.0e9

IN_DT = mybir.dt.float32r  # matmul operand storage dtype


def _build_program():
    nc = bacc.Bacc("TRN2")
    f32 = mybir.dt.float32
    EXP = mybir.ActivationFunctionType.Exp

    xT = nc.dram_tensor("xT", [C, T], IN_DT, kind="ExternalInput").ap()
    wqk = nc.dram_tensor("wqk", [C, 2 * HS], IN_DT, kind="ExternalInput").ap()
    wv = nc.dram_tensor("wv", [C, HS], IN_DT, kind="ExternalInput").ap()
    mask_d = nc.dram_tensor("mask", [P, MASK_W], f32, kind="ExternalInput").ap()
    pmask_d = nc.dram_tensor("pmask", [HS, CH], mybir.dt.uint8, kind="ExternalInput").ap()
    ident_d = nc.dram_tensor("ident", [P, P], f32, kind="ExternalInput").ap()
    out_d = nc.dram_tensor("out", [NSLOT * CH, HS], f32, kind="ExternalOutput").ap()

    xT_r = xT.rearrange("(co ci) t -> ci co t", ci=P)      # [128, 8, 4096]
    wqk_r = wqk.rearrange("(co ci) m -> ci co m", ci=P)    # [128, 8, 128]
    wv_r = wv.rearrange("(co ci) m -> ci co m", ci=P)      # [128, 8, 64]

    with tile.TileContext(nc) as tc:
        with (
            tc.tile_pool(name="const", bufs=1) as const_pool,
            tc.tile_pool(name="persist", bufs=1) as persist,
            tc.tile_pool(name="xin", bufs=6) as xpool,
            tc.tile_pool(name="vt", bufs=2) as vt_pool,
            tc.tile_pool(name="pt", bufs=3) as pt_pool,
            tc.tile_pool(name="osb", bufs=2) as osb_pool,
            tc.tile_pool(name="fin", bufs=3) as fin_pool,
            tc.tile_pool(name="proj_ps", bufs=2, space="PSUM") as proj_ps,
            tc.tile_pool(name="st_ps", bufs=2, space="PSUM") as st_ps,
            tc.tile_pool(name="ot_ps", bufs=2, space="PSUM") as ot_ps,
        ):
            # ---- constants / persistent state ----
            wqk_sb = const_pool.tile([P, CSUB, 2 * HS], IN_DT)
            wv_sb = const_pool.tile([P, CSUB, HS], IN_DT)
            mask_sb = const_pool.tile([P, MASK_W], f32)
            pmask_sb = const_pool.tile([HS, CH], mybir.dt.uint8)
            ident_sb = const_pool.tile([P, P], f32)
            nc.sync.dma_start(wqk_sb[:], wqk_r)
            nc.sync.dma_start(wv_sb[:], wv_r)
            nc.sync.dma_start(mask_sb[:], mask_d)
            nc.sync.dma_start(pmask_sb[:], pmask_d)
            nc.sync.dma_start(ident_sb[:], ident_d)

            kt_all = persist.tile([HS, T], IN_DT)            # K^T
            qt_stage = persist.tile([HS, NSLOT, CH], f32)    # Q^T select staging
            qt_slot = persist.tile([HS, NSLOT, CH], IN_DT)   # owned Q^T per slot
            v_all = persist.tile([P, NKT, HS + 1], IN_DT)    # V with ones column
            # 0x3F800000 = 1.0f; memset can't target float32r directly
            nc.vector.memset(
                v_all[:, :, HS : HS + 1].bitcast(mybir.dt.uint32), 0x3F800000
            )

            # ---- streamed projection + attention ----
            for c in range(NCH):
                xc = xpool.tile([P, CSUB, CH], IN_DT, tag="xc")
                nc.sync.dma_start(xc[:], xT_r[:, :, c * CH : (c + 1) * CH])

                # Q^T (rows 0:64) and K^T (rows 64:128), stacked projection
                qk_ps = proj_ps.tile([P, CH], f32, tag="proj")
                for cs in range(CSUB):
                    nc.tensor.matmul(
                        qk_ps[:],
                        lhsT=wqk_sb[:, cs, :],
                        rhs=xc[:, cs, :],
                        start=(cs == 0),
                        stop=(cs == CSUB - 1),
                    )
                nc.vector.tensor_copy(kt_all[:, c * CH : (c + 1) * CH], qk_ps[HS:P, :])
                j_dst = c // 2
                if c % 2 == 0:
                    nc.vector.tensor_copy(qt_stage[:, j_dst, :], qk_ps[0:HS, :])
                else:
                    nc.vector.copy_predicated(qt_stage[:, j_dst, :], pmask_sb[:], qk_ps[0:HS, :])
                    nc.vector.tensor_copy(qt_slot[:, j_dst, :], qt_stage[:, j_dst, :])

                # V natural ([t, h]) via x^T blocks as stationary operand
                v_ps = proj_ps.tile([P, KT_PER_CH, HS], f32, tag="proj")
                for tt in range(KT_PER_CH):
                    for cs in range(CSUB):
                        nc.tensor.matmul(
                            v_ps[:, tt, :],
                            lhsT=xc[:, cs, tt * P : (tt + 1) * P],
                            rhs=wv_sb[:, cs, :],
                            start=(cs == 0),
                            stop=(cs == CSUB - 1),
                        )
                nc.vector.tensor_copy(
                    v_all[:, c * KT_PER_CH : (c + 1) * KT_PER_CH, 0:HS], v_ps[:]
                )

                # At odd chunks, slot j = (c-1)//2 has its Q (and all the keys
                # of its causal range, which ends at this chunk): flush its
                # whole attention row, then finalize and release the PSUM bank.
                if c % 2 == 0:
                    continue
                j = (c - 1) // 2
                nk = 8 * j + 8
                ot = ot_ps.tile([P, CH], f32, tag="ot")
                for kt in range(nk):
                    st = st_ps.tile([P, CH], f32, tag="st")
                    nc.tensor.matmul(
                        st[:],
                        lhsT=kt_all[:, kt * P : (kt + 1) * P],
                        rhs=qt_slot[:, j, :],
                        start=True,
                        stop=True,
                    )
                    if kt >= 8 * j:  # within masked band of this slot
                        s2 = P * (8 * j + 7 - kt)
                        nc.vector.tensor_add(st[:], st[:], mask_sb[:, s2 : s2 + CH])
                    pt = pt_pool.tile([P, CH], IN_DT, tag="pt")
                    nc.scalar.activation(pt[:], st[:], EXP, scale=float(HS) ** -0.5)
                    nc.tensor.matmul(
                        ot[0 : HS + 1, :],
                        lhsT=v_all[:, kt, :],
                        rhs=pt[:],
                        start=(kt == 0),
                        stop=(kt == nk - 1),
                    )

                # finalize slot j: transpose back, normalize, store
                o_sb = osb_pool.tile([HS + 1, CH], f32, tag="osb")
                nc.scalar.copy(o_sb[:], ot[0 : HS + 1, :])
                for tt in range(KT_PER_CH):
                    tr = st_ps.tile([P, CH], f32, tag="st")  # only [:, :HS+1] used
                    nc.tensor.transpose(
                        tr[:, 0 : HS + 1],
                        o_sb[:, tt * P : (tt + 1) * P],
                        ident_sb[0 : HS + 1, 0 : HS + 1],
                    )
                    rec = fin_pool.tile([P, 1], f32, tag="rec")
                    nc.vector.reciprocal(rec[:], tr[:, HS : HS + 1])
                    fo = fin_pool.tile([P, HS], f32, tag="fo")
                    nc.vector.tensor_scalar_mul(fo[:], tr[:, 0:HS], rec[:])
                    r0 = j * CH + tt * P
                    nc.sync.dma_start(out_d[r0 : r0 + P, :], fo[:])

    nc.compile()
    return nc


_CACHE = {}


def _get_program():
    if "nc" not in _CACHE:
        _CACHE["nc"] = _build_program()
    return _CACHE["nc"]


def _host_inputs(x, Wk, Wq, Wv):
    x = np.asarray(x, dtype=np.float32)
    wqk = np.ascontiguousarray(
        np.concatenate([np.asarray(Wq), np.asarray(Wk)], axis=1), dtype=np.float32
    )
    wv = np.ascontiguousarray(np.asarray(Wv), dtype=np.float32)
    ident = np.eye(P, dtype=np.float32)

    xT = [np.ascontiguousarray(x[b].T) for b in range(B)]

    # mask[i, c] = 0 if c >= i + (896 - 512 p) else NEG
    ii = np.arange(P)[:, None]
    cc = np.arange(MASK_W)[None, :]
    masks = [
        np.where(cc >= ii + (896 - 512 * p), 0.0, NEG).astype(np.float32)
        for p in range(2)
    ]
    pmasks = [np.full((HS, CH), p, dtype=np.uint8) for p in range(2)]

    in_maps = []
    for core in range(2 * B):
        b, p = core // 2, core % 2
        in_maps.append(
            {
                "xT": xT[b],
                "wqk": wqk,
                "wv": wv,
                "mask": masks[p],
                "pmask": pmasks[p],
                "ident": ident,
            }
        )
    return in_maps


def _assemble(results):
    out = np.empty((B, T, HS), dtype=np.float32)
    for core in range(2 * B):
        b, p = core // 2, core % 2
        oc = results[core]["out"]
        for j in range(NSLOT):
            g = 2 * j + p
            out[b, g * CH : (g + 1) * CH, :] = oc[j * CH : (j + 1) * CH, :]
    return out


def run(x, Wk, Wq, Wv, trace=False):
    nc = _get_program()
    in_maps = _host_inputs(x, Wk, Wq, Wv)
    res = run_bass_kernel_spmd(nc, in_maps, list(range(2 * B)), trace=trace)
    return _assemble(res.results), res


def kernel(x, Wk, Wq, Wv):
    out, _ = run(x, Wk, Wq, Wv)
    return out



# revision 2
# speedup vs baseline: 2.5640x; 2.5640x over previous
"""Causal single-head attention on 8 Trainium2 NeuronCores.

Problem: x[4, 4096, 1024], Wq/Wk/Wv[1024, 64] ->
  out = softmax(causal(Q K^T / 8)) V   per batch, fp32.

Sharding: core i handles batch b = i//2 with query-chunk parity p = i%2
(512-wide query chunks; core p owns global chunks {p, 2+p, 4+p, 6+p}).
Both cores of a pair load the full x[b] (transposed on host to [C, T]) and
compute full K/V; causal work is balanced by interleaving query chunks.

All matmul operands are bf16 (fp32 accumulate in PSUM): halves HBM traffic,
runs the PE at 1 cycle/row (the fp32 path is 4), and enables FWL weight
loads. Scores are computed transposed (S^T[k, q]) with contraction HS=64,
so two key tiles are packed concurrently in the PE array via partition
row-groups (tiles at base partition 0 and 64). exp runs once per key-tile
pair over [128, 1024] spanning two PSUM banks. Causality is applied as a
multiplicative 0/1 bf16 mask after exp (host-built maskP gives both pair
halves in one strided access). V carries a ones column so softmax row-sums
accumulate in the same PSUM tile as P@V; the unnormalized out^T [65, q] is
DMA'd out and the division + transpose happen on host.
"""

import numpy as np
import ml_dtypes

import concourse.bacc as bacc
import concourse.mybir as mybir
import concourse.tile as tile
from concourse.bass_utils import run_bass_kernel_spmd

# Problem dims
B, T, C, HS = 4, 4096, 1024, 64
P = 128           # partitions
CH = 512          # query-chunk width
CHP = 2 * CH      # chunk-pair width (one DMA)
NCH = T // CH     # 8 chunks
NSLOT = NCH // 2  # 4 local query slots per core
CSUB = C // P     # 8 contraction subtiles
NKT = T // P      # 32 key tiles total
NPAIR = NKT // 2  # 16 key-tile pairs
MASK_W = 896 + CH  # mask window width

BF16 = mybir.dt.bfloat16


def _build_program():
    nc = bacc.Bacc("TRN2")
    f32 = mybir.dt.float32
    EXP = mybir.ActivationFunctionType.Exp

    xT = nc.dram_tensor("xT", [C, T], BF16, kind="ExternalInput").ap()
    wqk = nc.dram_tensor("wqk", [C, 2 * HS], BF16, kind="ExternalInput").ap()
    wv = nc.dram_tensor("wv", [C, HS], BF16, kind="ExternalInput").ap()
    maskp_d = nc.dram_tensor("maskp", [P, 2, MASK_W], BF16, kind="ExternalInput").ap()
    pmask_d = nc.dram_tensor("pmask", [HS, CH], mybir.dt.uint8, kind="ExternalInput").ap()
    out_d = nc.dram_tensor("out", [HS + 1, NSLOT * CH], f32, kind="ExternalOutput").ap()

    xT_r = xT.rearrange("(co ci) t -> ci co t", ci=P)      # [128, 8, 4096]
    wqk_r = wqk.rearrange("(co ci) m -> ci co m", ci=P)    # [128, 8, 128]
    wv_r = wv.rearrange("(co ci) m -> ci co m", ci=P)      # [128, 8, 64]

    with tile.TileContext(nc) as tc:
        with (
            tc.tile_pool(name="const", bufs=1) as const_pool,
            tc.tile_pool(name="persist", bufs=1) as persist,
            tc.tile_pool(name="xin", bufs=3) as xpool,
            tc.tile_pool(name="pt", bufs=3) as pt_pool,
            tc.tile_pool(name="osb", bufs=2) as osb_pool,
            tc.tile_pool(name="proj_ps", bufs=2, space="PSUM") as proj_ps,
            tc.tile_pool(name="st_ps", bufs=2, space="PSUM") as st_ps,
            tc.tile_pool(name="ot_ps", bufs=2, space="PSUM") as ot_ps,
        ):
            # ---- constants / persistent state ----
            wqk_sb = const_pool.tile([P, CSUB, 2 * HS], BF16)
            wv_sb = const_pool.tile([P, CSUB, HS], BF16)
            maskp_sb = const_pool.tile([P, 2, MASK_W], BF16)
            pmask_sb = const_pool.tile([HS, CH], mybir.dt.uint8)
            nc.sync.dma_start(wqk_sb[:], wqk_r)
            nc.sync.dma_start(wv_sb[:], wv_r)
            nc.sync.dma_start(maskp_sb[:], maskp_d)
            nc.sync.dma_start(pmask_sb[:], pmask_d)

            # K^T pairs: [0:64, u, :] = tile 2u, [64:128, u, :] = tile 2u+1
            kt_all = persist.tile([P, NPAIR, P], BF16)
            qt_stage = persist.tile([HS, NSLOT, CH], f32)    # Q^T select staging
            qt_slot = persist.tile([P, NSLOT, CH], BF16)     # Q^T dup'd both halves
            v_all = persist.tile([P, NKT, HS + 1], BF16)     # V with ones column
            nc.vector.memset(
                v_all[:, :, HS : HS + 1].bitcast(mybir.dt.uint16), 0x3F80
            )

            # ---- streamed projection + attention ----
            for cp in range(NSLOT):  # chunk pairs (even, odd)
                xc = xpool.tile([P, CSUB, CHP], BF16, tag="xc")
                nc.sync.dma_start(xc[:], xT_r[:, :, cp * CHP : (cp + 1) * CHP])

                for half in range(2):
                    c = 2 * cp + half
                    lo = half * CH
                    # Q^T (rows 0:64) and K^T (rows 64:128), stacked projection
                    qk_ps = proj_ps.tile([P, CH], f32, tag="proj")
                    for cs in range(CSUB):
                        nc.tensor.matmul(
                            qk_ps[:],
                            lhsT=wqk_sb[:, cs, :],
                            rhs=xc[:, cs, lo : lo + CH],
                            start=(cs == 0),
                            stop=(cs == CSUB - 1),
                        )
                    # chunk c holds key tiles 4c..4c+3 = pairs 2c, 2c+1
                    ksrc = qk_ps[HS:P, :].rearrange(
                        "p (i par c) -> p i par c", i=2, par=2, c=P
                    )
                    nc.vector.tensor_copy(
                        kt_all[0:HS, 2 * c : 2 * c + 2, :], ksrc[:, :, 0, :]
                    )
                    nc.vector.tensor_copy(
                        kt_all[HS:P, 2 * c : 2 * c + 2, :], ksrc[:, :, 1, :]
                    )
                    if half == 0:
                        nc.vector.tensor_copy(qt_stage[:, cp, :], qk_ps[0:HS, :])
                    else:
                        nc.vector.copy_predicated(
                            qt_stage[:, cp, :], pmask_sb[:], qk_ps[0:HS, :]
                        )
                        nc.vector.tensor_copy(qt_slot[0:HS, cp, :], qt_stage[:, cp, :])
                        nc.vector.tensor_copy(qt_slot[HS:P, cp, :], qt_stage[:, cp, :])

                    # V natural ([t, h]) via x^T blocks as stationary operand
                    v_ps = proj_ps.tile([P, 4, HS], f32, tag="proj")
                    for tt in range(4):
                        for cs in range(CSUB):
                            nc.tensor.matmul(
                                v_ps[:, tt, :],
                                lhsT=xc[:, cs, lo + tt * P : lo + (tt + 1) * P],
                                rhs=wv_sb[:, cs, :],
                                start=(cs == 0),
                                stop=(cs == CSUB - 1),
                            )
                    nc.vector.tensor_copy(
                        v_all[:, 4 * c : 4 * c + 4, 0:HS], v_ps[:]
                    )

                # slot j = cp owns global chunk 2j+p; flush its attention row
                j = cp
                npair = 4 * j + 4
                ot = ot_ps.tile([P, CH], f32, tag="ot")
                for u in range(npair):
                    st = st_ps.tile([P, 2, CH], f32, tag="st")
                    # half 0 <- key tile 2u+1 (base partition 64),
                    # half 1 <- key tile 2u (base partition 0)
                    nc.tensor.matmul(
                        st[:, 0, :],
                        lhsT=kt_all[HS:P, u, :],
                        rhs=qt_slot[HS:P, j, :],
                        start=True,
                        stop=True,
                    )
                    nc.tensor.matmul(
                        st[:, 1, :],
                        lhsT=kt_all[0:HS, u, :],
                        rhs=qt_slot[0:HS, j, :],
                        start=True,
                        stop=True,
                    )
                    pt = pt_pool.tile([P, 2, CH], BF16, tag="pt")
                    nc.scalar.activation(pt[:], st[:], EXP, scale=float(HS) ** -0.5)
                    if u >= 4 * j:  # masked band of this slot
                        s2 = P * (8 * j + 6 - 2 * u)
                        nc.vector.tensor_mul(
                            pt[:], pt[:], maskp_sb[:, :, s2 : s2 + CH]
                        )
                    nc.tensor.matmul(
                        ot[0 : HS + 1, :],
                        lhsT=v_all[:, 2 * u + 1, :],
                        rhs=pt[:, 0, :],
                        start=(u == 0),
                        stop=False,
                    )
                    nc.tensor.matmul(
                        ot[0 : HS + 1, :],
                        lhsT=v_all[:, 2 * u, :],
                        rhs=pt[:, 1, :],
                        start=False,
                        stop=(u == npair - 1),
                    )

                # store unnormalized out^T + sums row; host divides/transposes
                o_sb = osb_pool.tile([HS + 1, CH], f32, tag="osb")
                nc.vector.tensor_copy(o_sb[:], ot[0 : HS + 1, :])
                nc.sync.dma_start(out_d[:, j * CH : (j + 1) * CH], o_sb[:])

    nc.compile()
    return nc


_CACHE = {}


def _get_program():
    if "nc" not in _CACHE:
        _CACHE["nc"] = _build_program()
    return _CACHE["nc"]


def _host_inputs(x, Wk, Wq, Wv):
    bf = ml_dtypes.bfloat16
    x = np.asarray(x, dtype=np.float32)
    wqk = np.ascontiguousarray(
        np.concatenate([np.asarray(Wq), np.asarray(Wk)], axis=1), dtype=np.float32
    ).astype(bf)
    wv = np.ascontiguousarray(np.asarray(Wv), dtype=np.float32).astype(bf)

    xT = [np.ascontiguousarray(x[b].T).astype(bf) for b in range(B)]

    # maskp[i, h, c] = 1 if (c + 128 h) >= i + (896 - 512 p) else 0
    ii = np.arange(P)[:, None, None]
    hh = np.arange(2)[None, :, None]
    cc = np.arange(MASK_W)[None, None, :]
    maskps = [
        ((cc + P * hh) >= (ii + (896 - 512 * p))).astype(bf) for p in range(2)
    ]
    pmasks = [np.full((HS, CH), p, dtype=np.uint8) for p in range(2)]

    in_maps = []
    for core in range(2 * B):
        b, p = core // 2, core % 2
        in_maps.append(
            {
                "xT": xT[b],
                "wqk": wqk,
                "wv": wv,
                "maskp": maskps[p],
                "pmask": pmasks[p],
            }
        )
    return in_maps


def _assemble(results):
    out = np.empty((B, T, HS), dtype=np.float32)
    for core in range(2 * B):
        b, p = core // 2, core % 2
        oc = np.asarray(results[core]["out"], dtype=np.float32)  # [65, 2048]
        for j in range(NSLOT):
            g = 2 * j + p
            blk = oc[:, j * CH : (j + 1) * CH]
            out[b, g * CH : (g + 1) * CH, :] = (blk[0:HS] / blk[HS : HS + 1]).T
    return out


def run(x, Wk, Wq, Wv, trace=False):
    nc = _get_program()
    in_maps = _host_inputs(x, Wk, Wq, Wv)
    res = run_bass_kernel_spmd(nc, in_maps, list(range(2 * B)), trace=trace)
    return _assemble(res.results), res


def kernel(x, Wk, Wq, Wv):
    out, _ = run(x, Wk, Wq, Wv)
    return out


# revision 4
# speedup vs baseline: 2.5671x; 1.0012x over previous
"""Causal single-head attention on 8 Trainium2 NeuronCores.

Problem: x[4, 4096, 1024], Wq/Wk/Wv[1024, 64] ->
  out = softmax(causal(Q K^T / 8)) V   per batch, fp32.

Sharding: core i handles batch b = i//2 with query-chunk parity p = i%2
(512-wide query chunks; core p owns global chunks {p, 2+p, 4+p, 6+p}).
Both cores of a pair load the full x[b] (transposed on host to [C, T]) and
compute full K/V; causal work is balanced by interleaving query chunks.

All matmul operands are bf16 (fp32 accumulate in PSUM): halves HBM traffic,
runs the PE at 1 cycle/row (the fp32 path is 4), and enables FWL weight
loads. Scores are computed transposed (S^T[k, q]) with contraction HS=64,
so two key tiles are packed concurrently in the PE array via partition
row-groups (tiles at base partition 0 and 64). exp runs once per key-tile
pair over [128, 1024] spanning two PSUM banks. Causality is applied as a
multiplicative 0/1 bf16 mask after exp (host-built maskP gives both pair
halves in one strided access). V carries a ones column so softmax row-sums
accumulate in the same PSUM tile as P@V; the unnormalized out^T [65, q] is
DMA'd out and the division + transpose happen on host.
"""

import numpy as np
import ml_dtypes

import concourse.bacc as bacc
import concourse.mybir as mybir
import concourse.tile as tile
from concourse.bass_utils import run_bass_kernel_spmd

# Problem dims
B, T, C, HS = 4, 4096, 1024, 64
P = 128           # partitions
CH = 512          # query-chunk width
CHP = 2 * CH      # chunk-pair width (one DMA)
NCH = T // CH     # 8 chunks
NSLOT = NCH // 2  # 4 local query slots per core
CSUB = C // P     # 8 contraction subtiles
NKT = T // P      # 32 key tiles total
NPAIR = NKT // 2  # 16 key-tile pairs
MASK_W = 896 + CH  # mask window width

BF16 = mybir.dt.bfloat16


def _build_program():
    nc = bacc.Bacc("TRN2")
    f32 = mybir.dt.float32
    EXP = mybir.ActivationFunctionType.Exp

    xT = nc.dram_tensor("xT", [C, T], BF16, kind="ExternalInput").ap()
    wqk = nc.dram_tensor("wqk", [C, 2 * HS], BF16, kind="ExternalInput").ap()
    wv = nc.dram_tensor("wv", [C, HS], BF16, kind="ExternalInput").ap()
    maskp_d = nc.dram_tensor("maskp", [P, 2, MASK_W], BF16, kind="ExternalInput").ap()
    pmask_d = nc.dram_tensor("pmask", [HS, CH], mybir.dt.uint8, kind="ExternalInput").ap()
    out_d = nc.dram_tensor("out", [HS + 1, NSLOT * CH], f32, kind="ExternalOutput").ap()

    xT_r = xT.rearrange("(co ci) t -> ci co t", ci=P)      # [128, 8, 4096]
    wqk_r = wqk.rearrange("(co ci) m -> ci co m", ci=P)    # [128, 8, 128]
    wv_r = wv.rearrange("(co ci) m -> ci co m", ci=P)      # [128, 8, 64]

    with tile.TileContext(nc) as tc:
        with (
            tc.tile_pool(name="const", bufs=1) as const_pool,
            tc.tile_pool(name="persist", bufs=1) as persist,
            tc.tile_pool(name="xin", bufs=3) as xpool,
            tc.tile_pool(name="pt", bufs=3) as pt_pool,
            tc.tile_pool(name="osb", bufs=2) as osb_pool,
            tc.tile_pool(name="proj_ps", bufs=2, space="PSUM") as proj_ps,
            tc.tile_pool(name="st_ps", bufs=2, space="PSUM") as st_ps,
            tc.tile_pool(name="ot_ps", bufs=2, space="PSUM") as ot_ps,
        ):
            # ---- constants / persistent state ----
            # const loads go on the (otherwise idle) GpSimd queue so they
            # don't serialize ahead of the first x chunk on the Sync queue
            wqk_sb = const_pool.tile([P, CSUB, 2 * HS], BF16)
            wv_sb = const_pool.tile([P, CSUB, HS], BF16)
            maskp_sb = const_pool.tile([P, 2, MASK_W], BF16)
            pmask_sb = const_pool.tile([HS, CH], mybir.dt.uint8)
            nc.gpsimd.dma_start(wqk_sb[:], wqk_r)
            nc.gpsimd.dma_start(wv_sb[:], wv_r)
            nc.gpsimd.dma_start(maskp_sb[:], maskp_d)
            nc.gpsimd.dma_start(pmask_sb[:], pmask_d)

            # K^T pairs: [0:64, u, :] = tile 2u, [64:128, u, :] = tile 2u+1
            kt_all = persist.tile([P, NPAIR, P], BF16)
            qt_stage = persist.tile([HS, NSLOT, CH], f32)    # Q^T select staging
            qt_slot = persist.tile([P, NSLOT, CH], BF16)     # Q^T dup'd both halves
            v_all = persist.tile([P, NKT, HS + 1], BF16)     # V with ones column
            nc.vector.memset(
                v_all[:, :, HS : HS + 1].bitcast(mybir.dt.uint16), 0x3F80
            )

            # ---- streamed projection + attention ----
            for cp in range(NSLOT):  # chunk pairs (even, odd)
                xc = xpool.tile([P, CSUB, CHP], BF16, tag="xc")
                if cp == 0:
                    # split so the first chunk's projection starts sooner
                    nc.sync.dma_start(xc[:, :, 0:CH], xT_r[:, :, 0:CH])
                    nc.sync.dma_start(xc[:, :, CH:CHP], xT_r[:, :, CH:CHP])
                else:
                    nc.sync.dma_start(xc[:], xT_r[:, :, cp * CHP : (cp + 1) * CHP])

                for half in range(2):
                    c = 2 * cp + half
                    lo = half * CH
                    # Q^T (rows 0:64) and K^T (rows 64:128), stacked projection
                    qk_ps = proj_ps.tile([P, CH], f32, tag="proj")
                    for cs in range(CSUB):
                        nc.tensor.matmul(
                            qk_ps[:],
                            lhsT=wqk_sb[:, cs, :],
                            rhs=xc[:, cs, lo : lo + CH],
                            start=(cs == 0),
                            stop=(cs == CSUB - 1),
                        )
                    # chunk c holds key tiles 4c..4c+3 = pairs 2c, 2c+1
                    ksrc = qk_ps[HS:P, :].rearrange(
                        "p (i par c) -> p i par c", i=2, par=2, c=P
                    )
                    nc.vector.tensor_copy(
                        kt_all[0:HS, 2 * c : 2 * c + 2, :], ksrc[:, :, 0, :]
                    )
                    nc.vector.tensor_copy(
                        kt_all[HS:P, 2 * c : 2 * c + 2, :], ksrc[:, :, 1, :]
                    )
                    if half == 0:
                        nc.vector.tensor_copy(qt_stage[:, cp, :], qk_ps[0:HS, :])
                    else:
                        nc.vector.copy_predicated(
                            qt_stage[:, cp, :], pmask_sb[:], qk_ps[0:HS, :]
                        )
                        nc.vector.tensor_copy(qt_slot[0:HS, cp, :], qt_stage[:, cp, :])
                        nc.vector.tensor_copy(qt_slot[HS:P, cp, :], qt_stage[:, cp, :])

                    # V natural ([t, h]) via x^T blocks as stationary operand
                    v_ps = proj_ps.tile([P, 4, HS], f32, tag="proj")
                    for tt in range(4):
                        for cs in range(CSUB):
                            nc.tensor.matmul(
                                v_ps[:, tt, :],
                                lhsT=xc[:, cs, lo + tt * P : lo + (tt + 1) * P],
                                rhs=wv_sb[:, cs, :],
                                start=(cs == 0),
                                stop=(cs == CSUB - 1),
                            )
                    nc.vector.tensor_copy(
                        v_all[:, 4 * c : 4 * c + 4, 0:HS], v_ps[:]
                    )

                # slot j = cp owns global chunk 2j+p; flush its attention row
                j = cp
                npair = 4 * j + 4
                ot = ot_ps.tile([P, CH], f32, tag="ot")
                for u in range(npair):
                    st = st_ps.tile([P, 2, CH], f32, tag="st")
                    # half 0 <- key tile 2u+1 (base partition 64),
                    # half 1 <- key tile 2u (base partition 0)
                    nc.tensor.matmul(
                        st[:, 0, :],
                        lhsT=kt_all[HS:P, u, :],
                        rhs=qt_slot[HS:P, j, :],
                        start=True,
                        stop=True,
                    )
                    nc.tensor.matmul(
                        st[:, 1, :],
                        lhsT=kt_all[0:HS, u, :],
                        rhs=qt_slot[0:HS, j, :],
                        start=True,
                        stop=True,
                    )
                    pt = pt_pool.tile([P, 2, CH], BF16, tag="pt")
                    nc.scalar.activation(pt[:], st[:], EXP, scale=float(HS) ** -0.5)
                    if u >= 4 * j:  # masked band of this slot
                        s2 = P * (8 * j + 6 - 2 * u)
                        nc.vector.tensor_mul(
                            pt[:], pt[:], maskp_sb[:, :, s2 : s2 + CH]
                        )
                    nc.tensor.matmul(
                        ot[0 : HS + 1, :],
                        lhsT=v_all[:, 2 * u + 1, :],
                        rhs=pt[:, 0, :],
                        start=(u == 0),
                        stop=False,
                    )
                    nc.tensor.matmul(
                        ot[0 : HS + 1, :],
                        lhsT=v_all[:, 2 * u, :],
                        rhs=pt[:, 1, :],
                        start=False,
                        stop=(u == npair - 1),
                    )

                # store unnormalized out^T + sums row; host divides/transposes
                o_sb = osb_pool.tile([HS + 1, CH], f32, tag="osb")
                nc.vector.tensor_copy(o_sb[:], ot[0 : HS + 1, :])
                nc.sync.dma_start(out_d[:, j * CH : (j + 1) * CH], o_sb[:])

    nc.compile()
    return nc


_CACHE = {}


def _get_program():
    if "nc" not in _CACHE:
        _CACHE["nc"] = _build_program()
    return _CACHE["nc"]


def _host_inputs(x, Wk, Wq, Wv):
    bf = ml_dtypes.bfloat16
    x = np.asarray(x, dtype=np.float32)
    wqk = np.ascontiguousarray(
        np.concatenate([np.asarray(Wq), np.asarray(Wk)], axis=1), dtype=np.float32
    ).astype(bf)
    wv = np.ascontiguousarray(np.asarray(Wv), dtype=np.float32).astype(bf)

    xT = [np.ascontiguousarray(x[b].T).astype(bf) for b in range(B)]

    # maskp[i, h, c] = 1 if (c + 128 h) >= i + (896 - 512 p) else 0
    ii = np.arange(P)[:, None, None]
    hh = np.arange(2)[None, :, None]
    cc = np.arange(MASK_W)[None, None, :]
    maskps = [
        ((cc + P * hh) >= (ii + (896 - 512 * p))).astype(bf) for p in range(2)
    ]
    pmasks = [np.full((HS, CH), p, dtype=np.uint8) for p in range(2)]

    in_maps = []
    for core in range(2 * B):
        b, p = core // 2, core % 2
        in_maps.append(
            {
                "xT": xT[b],
                "wqk": wqk,
                "wv": wv,
                "maskp": maskps[p],
                "pmask": pmasks[p],
            }
        )
    return in_maps


def _assemble(results):
    out = np.empty((B, T, HS), dtype=np.float32)
    for core in range(2 * B):
        b, p = core // 2, core % 2
        oc = np.asarray(results[core]["out"], dtype=np.float32)  # [65, 2048]
        for j in range(NSLOT):
            g = 2 * j + p
            blk = oc[:, j * CH : (j + 1) * CH]
            out[b, g * CH : (g + 1) * CH, :] = (blk[0:HS] / blk[HS : HS + 1]).T
    return out


def run(x, Wk, Wq, Wv, trace=False):
    nc = _get_program()
    in_maps = _host_inputs(x, Wk, Wq, Wv)
    res = run_bass_kernel_spmd(nc, in_maps, list(range(2 * B)), trace=trace)
    return _assemble(res.results), res


def kernel(x, Wk, Wq, Wv):
    out, _ = run(x, Wk, Wq, Wv)
    return out
